# revision 2
# baseline (speedup 1.0000x reference)
"""BEVLiftNet Trainium2 kernel.

Sharding: 8 cores = 2 batches x 4 channel-groups (16 ch each).
Each core lifts all 4 cameras of its batch (depths -> voxel ids), then
scatter-adds 16-channel feature rows into per-core DRAM grids
[Z*X*Y rows, 16] via indirect DMA with CCE-add.  Duplicate voxel rows
within one 128-token chunk are pre-combined with a selection-matrix
matmul so every CCE call carries unique row indices (DMA CCE
read-modify-write races on in-flight duplicates); calls are ordered by
the tile framework's WAW semaphores.  Chunks alternate between two grid
replicas to overlap the serialized chains.  A final CCE pass sums the
replicas and max-reduces over Z on-device; the host reassembles the
[1, B*C, X, Y] output (flips + channel concat).
"""

import sys

sys.path.insert(0, "/opt/trn_rl_repo")

import numpy as np

import concourse.bacc as bacc
import concourse.bass as bass
import concourse.mybir as mybir
import concourse.tile as tile
from concourse import bass_utils
from concourse.masks import make_identity

B, N, C, H, W = 2, 4, 64, 128, 352
X, Y, Z = 256, 256, 8
CG = 16           # channels per core
NCAM = 4          # cameras per core (one batch)
HW = H * W        # 45056 pixels per camera
NTOK = NCAM * HW  # 180224 tokens per core
P = 128
FCAM = HW // P    # 352 chunk-columns per camera
F = NCAM * FCAM   # 1408 chunk-columns total
NROWS = Z * X * Y          # 524288 voxel rows per grid
TRASH = NROWS              # extra row for dropped tokens
f32 = mybir.dt.float32
i32 = mybir.dt.int32
AL = mybir.AluOpType
ACT = mybir.ActivationFunctionType

_prog_cache = None


def _build_program():
    nc = bacc.Bacc("TRN2", target_bir_lowering=False, debug=False)
    feat = nc.dram_tensor("feat", [NTOK, CG], f32, kind="ExternalInput")
    dep = nc.dram_tensor("dep", [NTOK], f32, kind="ExternalInput")
    kmat = nc.dram_tensor("kmat", [NCAM, 3, 3], f32, kind="ExternalInput")
    emat = nc.dram_tensor("emat", [NCAM, 4, 4], f32, kind="ExternalInput")
    tmat_t = nc.dram_tensor("tmat_t", [NCAM, 4, 4], f32, kind="ExternalInput")
    out = nc.dram_tensor("out", [X * Y, CG], f32, kind="ExternalOutput")
    grids = [nc.dram_tensor("grid0", [NROWS + 1, CG], f32,
                            kind="ExternalInput")]
    lin_dram = nc.dram_tensor("lin_dram", [F, P], f32, kind="Internal")
    with tile.TileContext(nc) as tc:
        _emit(tc, feat.ap(), dep.ap(), kmat.ap(), emat.ap(), tmat_t.ap(),
              out.ap(), [g.ap() for g in grids], lin_dram.ap())
    nc.compile()
    return nc


def _floor(nc, wp, out_t, in_t, n):
    """out = floor(in), robust to any f32->i32 rounding mode."""
    ii = wp.tile([P, n], i32, tag="fl_i")
    nc.vector.tensor_copy(ii[:], in_t)
    ff = wp.tile([P, n], f32, tag="fl_f")
    nc.vector.tensor_copy(ff[:], ii[:])
    gt = wp.tile([P, n], f32, tag="fl_g")
    nc.vector.tensor_tensor(out=gt[:], in0=ff[:], in1=in_t, op=AL.is_gt)
    nc.vector.tensor_tensor(out=out_t, in0=ff[:], in1=gt[:], op=AL.subtract)


def _emit(tc, feat, dep, kmat, emat, tmat_t, out, grids, lin_dram):
    import os
    _SKIP_ZERO = os.environ.get("KBEV_SKIP_ZERO")
    _SKIP_LIFT = os.environ.get("KBEV_SKIP_LIFT")
    _SKIP_MERGE = os.environ.get("KBEV_SKIP_MERGE")
    nc = tc.nc

    with tc.tile_pool(name="persist", bufs=1) as pp, \
         tc.tile_pool(name="psum", bufs=2, space="PSUM") as psp:

        # ---- zero the grids (HWDGE queues, overlaps with lift) ----
        with tc.tile_pool(name="zpool", bufs=1) as zp:
          if False:
            ztile = zp.tile([P, 2048], f32, tag="zz")
            nc.vector.memset(ztile[:], 0.0)
            whole = (NROWS + 1) * CG
            step = P * 2048
            for g in grids:
                gflat = g.rearrange("a c -> (a c)")
                starts = list(range(0, whole - step + 1, step))
                if starts[-1] + step < whole:
                    starts.append(whole - step)  # overlapped tail, re-zeroed
                for s in starts:
                    nc.sync.dma_start(
                        gflat[s:s + step].rearrange("(p m) -> p m", p=P),
                        ztile[:])

        ident = pp.tile([P, P], f32, tag="ident")
        make_identity(nc, ident[:])
        # lowmask[p, k] = 1.0 where k < p   (strictly lower along free dim)
        iot = pp.tile([P, P], i32, tag="iotpk")
        nc.gpsimd.iota(iot[:], pattern=[[1, P]], base=0, channel_multiplier=-1)
        lowmask = pp.tile([P, P], f32, tag="lowmask")
        nc.vector.tensor_scalar(out=lowmask[:], in0=iot[:], scalar1=0,
                                scalar2=None, op0=AL.is_lt)

        # ---- pixel-constant tiles (shared by all cameras) ----
        wp_cm = tc.tile_pool(name="work", bufs=2)
        wp = wp_cm.__enter__()
        idl = pp.tile([P, FCAM], i32, tag="idl")
        nc.gpsimd.iota(idl[:], pattern=[[1, FCAM]], base=0, channel_multiplier=FCAM)
        idf = pp.tile([P, FCAM], f32, tag="idf")
        nc.scalar.copy(idf[:], idl[:])
        yf = pp.tile([P, FCAM], f32, tag="yf")
        xf = pp.tile([P, FCAM], f32, tag="xf")
        tmp = wp.tile([P, FCAM], f32, tag="t0")
        nc.scalar.activation(tmp[:], idf[:], ACT.Copy, bias=0.5 / W, scale=1.0 / W)
        _floor(nc, wp, yf[:], tmp[:], FCAM)
        nc.vector.scalar_tensor_tensor(out=xf[:], in0=yf[:], scalar=-float(W),
                                       in1=idf[:], op0=AL.mult, op1=AL.add)

        # ---- per-camera geometry constants ----
        cam_consts = []
        for cam in range(NCAM):
            kc = pp.tile([3, 3], f32, tag=f"kc_{cam}")
            ec = pp.tile([4, 4], f32, tag=f"ec_{cam}")
            tmc = pp.tile([4, 4], f32, tag=f"tc_{cam}")
            nc.sync.dma_start(kc[:], kmat[cam])
            nc.sync.dma_start(ec[:], emat[cam])
            nc.sync.dma_start(tmc[:], tmat_t[cam])
            m4p = psp.tile([4, 4], f32, tag="smallp")
            nc.tensor.matmul(m4p[:], lhsT=tmc[:], rhs=ec[:],
                             start=True, stop=True)
            m4 = pp.tile([4, 4], f32, tag=f"m4_{cam}")
            nc.vector.tensor_copy(m4[:], m4p[:])
            m4tp = psp.tile([4, 4], f32, tag="smallp")
            nc.tensor.transpose(out=m4tp[:], in_=m4[:], identity=ident[:4, :4])
            m4t = pp.tile([4, 4], f32, tag=f"m4t_{cam}")
            nc.vector.tensor_copy(m4t[:], m4tp[:])
            n3p = psp.tile([3, 3], f32, tag="smallp")
            nc.tensor.matmul(n3p[:], lhsT=m4t[:3, :3], rhs=kc[:],
                             start=True, stop=True)
            n3 = pp.tile([3, 3], f32, tag=f"n3_{cam}")
            nc.vector.tensor_copy(n3[:], n3p[:])
            pk = pp.tile([1, 12], f32, tag=f"pk_{cam}")
            for i in range(3):
                nc.gpsimd.dma_start(pk[:, 3 * i:3 * i + 3], n3[i:i + 1, :])
                nc.gpsimd.dma_start(pk[:, 9 + i:10 + i], m4[i:i + 1, 3:4])
            bc = pp.tile([P, 12], f32, tag=f"bc_{cam}")
            nc.gpsimd.partition_broadcast(bc[:], pk[:])
            cam_consts.append(bc)

        # ---- load depths and features ----
        dtile = pp.tile([P, F], f32, tag="dtile")
        for cam in range(NCAM):
            cs = slice(cam * FCAM, (cam + 1) * FCAM)
            nc.sync.dma_start(
                dtile[:, cs],
                dep[cam * HW:(cam + 1) * HW].rearrange("(p f) -> p f", p=P))

        # ---- lift: voxel linear index per token ----
        linf = pp.tile([P, F], f32, tag="linf")
        nc.vector.memset(linf[:], float(TRASH))
        for cam in ([] if _SKIP_LIFT else list(range(NCAM))):
            cs = slice(cam * FCAM, (cam + 1) * FCAM)
            bc = cam_consts[cam]
            d = dtile[:, cs]
            gs = []
            for i in range(3):
                a = wp.tile([P, FCAM], f32, tag="a_i")
                nc.vector.tensor_scalar(out=a[:], in0=xf[:],
                                        scalar1=bc[:, 3 * i:3 * i + 1],
                                        scalar2=None, op0=AL.mult)
                nc.vector.scalar_tensor_tensor(out=a[:], in0=yf[:],
                                               scalar=bc[:, 3 * i + 1:3 * i + 2],
                                               in1=a[:], op0=AL.mult, op1=AL.add)
                nc.vector.tensor_scalar(out=a[:], in0=a[:],
                                        scalar1=bc[:, 3 * i + 2:3 * i + 3],
                                        scalar2=None, op0=AL.add)
                e = wp.tile([P, FCAM], f32, tag="e_i")
                nc.vector.tensor_tensor(out=e[:], in0=a[:], in1=d, op=AL.mult)
                nc.vector.tensor_scalar(out=e[:], in0=e[:],
                                        scalar1=bc[:, 9 + i:10 + i],
                                        scalar2=None, op0=AL.add)
                mid = (X / 2.0, Y / 2.0, Z / 2.0)[i]
                g = wp.tile([P, FCAM], f32, tag=f"g_{i}")
                nc.scalar.activation(g[:], e[:], ACT.Copy, bias=mid, scale=2.0)
                gs.append(g)
            gx, gy, gz = gs
            v = wp.tile([P, FCAM], f32, tag="v")
            nc.vector.tensor_scalar(out=v[:], in0=gx[:], scalar1=-1.0,
                                    scalar2=None, op0=AL.is_gt)
            nc.vector.scalar_tensor_tensor(out=v[:], in0=gx[:], scalar=float(X),
                                           in1=v[:], op0=AL.is_lt,
                                           op1=AL.logical_and)
            for gg, bound in ((gy, float(Y)), (gz, float(Z))):
                v2 = wp.tile([P, FCAM], f32, tag="v2")
                nc.vector.tensor_scalar(out=v2[:], in0=gg[:], scalar1=-1.0,
                                        scalar2=None, op0=AL.is_gt)
                nc.vector.scalar_tensor_tensor(out=v2[:], in0=gg[:], scalar=bound,
                                               in1=v2[:], op0=AL.is_lt,
                                               op1=AL.logical_and)
                nc.vector.tensor_tensor(out=v[:], in0=v[:], in1=v2[:],
                                        op=AL.logical_and)
            fx = wp.tile([P, FCAM], f32, tag="fx")
            fy = wp.tile([P, FCAM], f32, tag="fy")
            fz = wp.tile([P, FCAM], f32, tag="fz")
            _floor(nc, wp, fx[:], gx[:], FCAM)
            _floor(nc, wp, fy[:], gy[:], FCAM)
            _floor(nc, wp, fz[:], gz[:], FCAM)
            for ft in (fx, fy, fz):
                nc.vector.tensor_scalar(out=ft[:], in0=ft[:], scalar1=0.0,
                                        scalar2=255.0, op0=AL.max, op1=AL.min)
            lf = linf[:, cs]
            nc.vector.scalar_tensor_tensor(out=lf, in0=fz[:], scalar=float(X),
                                           in1=fx[:], op0=AL.mult, op1=AL.add)
            nc.vector.scalar_tensor_tensor(out=lf, in0=lf, scalar=float(Y),
                                           in1=fy[:], op0=AL.mult, op1=AL.add)
            # blend invalid -> TRASH :  lin = TRASH + v*(lin-TRASH)
            nc.vector.tensor_scalar(out=lf, in0=lf, scalar1=-float(TRASH),
                                    scalar2=None, op0=AL.add)
            nc.vector.tensor_tensor(out=lf, in0=lf, in1=v[:], op=AL.mult)
            nc.vector.tensor_scalar(out=lf, in0=lf, scalar1=float(TRASH),
                                    scalar2=None, op0=AL.add)

        # ---- stage lin columns to DRAM (chunk-major) ----
        NBLK = (F + P - 1) // P
        lin_t = pp.tile([P, NBLK, P], f32, tag="lin_t")
        for blk in range(NBLK):
            w = min(P, F - blk * P)
            ltp = psp.tile([P, P], f32, tag="tp")
            nc.tensor.transpose(out=ltp[:w, :], in_=linf[:, blk * P:blk * P + w],
                                identity=ident[:])
            nc.vector.tensor_copy(lin_t[:w, blk, :], ltp[:w, :])
            nc.sync.dma_start(lin_dram[blk * P:blk * P + w, :],
                              lin_t[:w, blk, :])

        # ---- scatter: per-chunk dedup + serialized RMW (2 replica chains) ----
        import os
        _FLIM = int(os.environ.get("KBEV_FLIM", F))

        def chunk_body(fidx):
            col_f = wp.tile([P, 1], f32, tag="colf")
            nc.sync.dma_start(col_f[:], lin_dram[fidx][:, None])
            payc = wp.tile([P, CG], f32, tag="payc")
            nc.sync.dma_start(payc[:], feat[bass.ts(fidx, P), :])
            tp = psp.tile([P, P], f32, tag="tp")
            nc.tensor.transpose(out=tp[:], in_=col_f[:].to_broadcast([P, P]),
                                identity=ident[:])
            rowv = wp.tile([P, P], f32, tag="rowv")
            nc.vector.tensor_copy(rowv[:], tp[:])
            eq = wp.tile([P, P], f32, tag="eq")
            nc.vector.tensor_tensor(out=eq[:], in0=col_f[:].to_broadcast([P, P]),
                                    in1=rowv[:], op=AL.is_equal)
            earlier = wp.tile([P, P], f32, tag="earlier")
            nc.vector.tensor_tensor(out=earlier[:], in0=eq[:], in1=lowmask[:],
                                    op=AL.mult)
            ecnt = wp.tile([P, 1], f32, tag="ecnt")
            nc.vector.tensor_reduce(out=ecnt[:], in_=earlier[:],
                                    axis=mybir.AxisListType.X, op=AL.add)
            first = wp.tile([P, 1], f32, tag="first")
            nc.vector.tensor_scalar(out=first[:], in0=ecnt[:], scalar1=0.0,
                                    scalar2=None, op0=AL.is_equal)
            totp = psp.tile([P, CG], f32, tag="totp")
            nc.tensor.matmul(totp[:], lhsT=eq[:], rhs=payc[:],
                             start=True, stop=True)
            tot = wp.tile([P, CG], f32, tag="tot")
            nc.vector.tensor_copy(tot[:], totp[:])
            df = wp.tile([P, 1], f32, tag="df")
            nc.vector.scalar_tensor_tensor(out=df[:], in0=col_f[:],
                                           scalar=-float(TRASH), in1=first[:],
                                           op0=AL.add, op1=AL.mult)
            nc.vector.tensor_scalar(out=df[:], in0=df[:], scalar1=float(TRASH),
                                    scalar2=None, op0=AL.add)
            di = wp.tile([P, 1], i32, tag="di")
            nc.vector.tensor_copy(di[:], df[:])
            cur = wp.tile([P, CG], f32, tag="cur")
            nc.gpsimd.indirect_dma_start(
                out=cur[:], out_offset=None, in_=grids[0],
                in_offset=bass.IndirectOffsetOnAxis(ap=di[:], axis=0))
            nc.vector.tensor_tensor(out=cur[:], in0=cur[:], in1=tot[:],
                                    op=AL.add)
            nc.gpsimd.indirect_dma_start(
                out=grids[0],
                out_offset=bass.IndirectOffsetOnAxis(ap=di[:], axis=0),
                in_=cur[:], in_offset=None)

        if _FLIM:
            tc.For_i_unrolled(0, _FLIM, 1, chunk_body, max_unroll=8)

        # ---- merge replicas + z-max + store ----
        wp_cm.__exit__(None, None, None)
        SL = X * Y
        HF = SL // P // 2  # half-slice free length (256)
        with tc.tile_pool(name="merge", bufs=1) as mp:
            for h in range(2):
                acc = mp.tile([P, HF, CG], f32, tag="acc")
                ta = mp.tile([P, HF, CG], f32, tag="ta")
                for z in range(Z):
                    rows = slice(z * SL + h * (SL // 2),
                                 z * SL + (h + 1) * (SL // 2))
                    a_sl = grids[0][rows, :].rearrange("(p f) c -> p f c", p=P)
                    nc.sync.dma_start(ta[:], a_sl)
                    if z == 0:
                        nc.vector.tensor_copy(acc[:], ta[:])
                    else:
                        nc.vector.tensor_tensor(out=acc[:], in0=acc[:],
                                                in1=ta[:], op=AL.max)
                orows = slice(h * (SL // 2), (h + 1) * (SL // 2))
                nc.sync.dma_start(
                    out[orows, :].rearrange("(p f) c -> p f c", p=P), acc[:])


def kernel(feat_maps, depths, K, E, T):
    global _prog_cache
    feat_maps = np.asarray(feat_maps, np.float32)
    depths = np.asarray(depths, np.float32)
    K = np.asarray(K, np.float32)
    E = np.asarray(E, np.float32)
    T = np.asarray(T, np.float32)

    if _prog_cache is None:
        _prog_cache = _build_program()
    nc = _prog_cache

    in_maps = []
    for core in range(8):
        b, cg = core // 4, core % 4
        ch = slice(cg * CG, (cg + 1) * CG)
        frows = np.concatenate([
            np.ascontiguousarray(
                feat_maps[b * N + n, ch].transpose(1, 2, 0).reshape(HW, CG)
                .reshape(P, FCAM, CG).transpose(1, 0, 2).reshape(HW, CG))
            for n in range(N)], axis=0)
        cams = slice(b * N, (b + 1) * N)
        zg = np.zeros((NROWS + 1, CG), np.float32)
        in_maps.append({
            "feat": frows,
            "dep": np.ascontiguousarray(depths[b].reshape(NTOK)),
            "kmat": np.ascontiguousarray(K[cams]),
            "emat": np.ascontiguousarray(E[cams]),
            "tmat_t": np.ascontiguousarray(T[cams].transpose(0, 2, 1)),
            "grid0": zg,
        })

    import os
    _tr = bool(os.environ.get("KBEV_TRACE"))
    res = bass_utils.run_bass_kernel_spmd(
        nc, in_maps, core_ids=list(range(8)), trace=_tr,
        trace_cores=(list(range(8)) if os.environ.get("KBEV_TRACE_ALL")
                     else [0]) if _tr else None)
    global last_result
    last_result = res
    outp = np.zeros((1, B * C, X, Y), np.float32)
    for core in range(8):
        b, cg = core // 4, core % 4
        o = res.results[core]["out"].reshape(X, Y, CG)
        outp[0, b * C + cg * CG:b * C + (cg + 1) * CG] = \
            o.transpose(2, 0, 1)[:, ::-1, ::-1]
    return outp



# revision 5
# speedup vs baseline: 1.9543x; 1.9543x over previous
"""BEVLiftNet Trainium2 kernel (V2: CCE-add scatter, replica chains).

Sharding: 8 cores = 2 batches x 4 channel-groups (16 ch each).
Each core lifts all 4 cameras of its batch (depths -> voxel ids), then
scatter-adds 16-channel feature rows into R replica DRAM grids via
indirect DMA with CCE-add (compute_op=add).  One call per 128-token
chunk; within-chunk duplicate rows are pre-combined with an eq-matrix
matmul and redirected to per-partition-unique trash rows, so every
CCE call carries unique row indices (concurrent CCE RMWs to the same
address lose updates -- measured).  Chunks round-robin over R replica
tensors so R WAW chains overlap and gpsimd issues calls back-to-back.
A final pass sums the replicas, max-reduces over Z, and stores
[X*Y, 16] f32; the host reassembles the [1, B*C, X, Y] output.
"""

import os
import sys

sys.path.insert(0, "/opt/trn_rl_repo")

import numpy as np

import concourse.bacc as bacc
import concourse.bass as bass
import concourse.mybir as mybir
import concourse.tile as tile
from concourse import bass_utils
from concourse.masks import make_identity

B, N, C, H, W = 2, 4, 64, 128, 352
X, Y, Z = 256, 256, 8
CG = 16           # channels per core
NCAM = 4          # cameras per core (one batch)
HW = H * W        # 45056 pixels per camera
NTOK = NCAM * HW  # 180224 tokens per core
P = 128
FCAM = HW // P    # 352 chunk-columns per camera
F = NCAM * FCAM   # 1408 chunk-columns total
NROWS = Z * X * Y          # 524288 voxel rows per grid
TRASH = NROWS              # row for first invalid token
GROWS = NROWS + 1 + P      # + shared trash + per-partition trash
f32 = mybir.dt.float32
bf16 = mybir.dt.bfloat16
i32 = mybir.dt.int32
AL = mybir.AluOpType
ACT = mybir.ActivationFunctionType

R = int(os.environ.get("KBEV_R", "4"))           # replica grids
GD = bf16 if os.environ.get("KBEV_GD", "bf16") == "bf16" else f32
UNROLL = int(os.environ.get("KBEV_UNROLL", "8"))

_prog_cache = None


def _build_program():
    nc = bacc.Bacc("TRN2", target_bir_lowering=False, debug=False)
    feat = nc.dram_tensor("feat", [NTOK, CG], f32, kind="ExternalInput")
    dep = nc.dram_tensor("dep", [NTOK], f32, kind="ExternalInput")
    kmat = nc.dram_tensor("kmat", [NCAM, 3, 3], f32, kind="ExternalInput")
    emat = nc.dram_tensor("emat", [NCAM, 4, 4], f32, kind="ExternalInput")
    tmat_t = nc.dram_tensor("tmat_t", [NCAM, 4, 4], f32, kind="ExternalInput")
    out = nc.dram_tensor("out", [X * Y, CG], f32, kind="ExternalOutput")
    grids = [nc.dram_tensor(f"grid{r}", [GROWS, CG], GD, kind="Internal")
             for r in range(R)]
    lin_dram = nc.dram_tensor("lin_dram", [F, P], f32, kind="Internal")
    with tile.TileContext(nc) as tc:
        _emit(tc, feat.ap(), dep.ap(), kmat.ap(), emat.ap(), tmat_t.ap(),
              out.ap(), [g.ap() for g in grids], lin_dram.ap())
    nc.compile()
    return nc


def _floor(nc, wp, out_t, in_t, n):
    """out = floor(in), robust to any f32->i32 rounding mode."""
    ii = wp.tile([P, n], i32, tag="fl_i")
    nc.vector.tensor_copy(ii[:], in_t)
    ff = wp.tile([P, n], f32, tag="fl_f")
    nc.vector.tensor_copy(ff[:], ii[:])
    gt = wp.tile([P, n], f32, tag="fl_g")
    nc.vector.tensor_tensor(out=gt[:], in0=ff[:], in1=in_t, op=AL.is_gt)
    nc.vector.tensor_tensor(out=out_t, in0=ff[:], in1=gt[:], op=AL.subtract)


def _emit(tc, feat, dep, kmat, emat, tmat_t, out, grids, lin_dram):
    nc = tc.nc

    with tc.tile_pool(name="persist", bufs=1) as pp, \
         tc.tile_pool(name="psum", bufs=2, space="PSUM") as psp:

        ident = pp.tile([P, P], f32, tag="ident")
        make_identity(nc, ident[:])
        # lowmask[p, k] = 1.0 where k < p   (strictly lower along free dim)
        iot = pp.tile([P, P], i32, tag="iotpk")
        nc.gpsimd.iota(iot[:], pattern=[[1, P]], base=0, channel_multiplier=-1)
        lowmask = pp.tile([P, P], f32, tag="lowmask")
        nc.vector.tensor_scalar(out=lowmask[:], in0=iot[:], scalar1=0,
                                scalar2=None, op0=AL.is_lt)
        # trash_col[p] = TRASH + 1 + p  (per-partition-unique trash rows)
        piot = pp.tile([P, 1], i32, tag="piot")
        nc.gpsimd.iota(piot[:], pattern=[[0, 1]], base=TRASH + 1,
                       channel_multiplier=1)
        trash_col = pp.tile([P, 1], f32, tag="trashc")
        nc.scalar.copy(trash_col[:], piot[:])

        # ---- zero the replica grids (HWDGE, overlaps with lift) ----
        with tc.tile_pool(name="zpool", bufs=1) as zp:
            ztile = zp.tile([P, 4096], GD, tag="zz")
            nc.vector.memset(ztile[:], 0.0)
            whole = GROWS * CG
            step = P * 4096
            for g in grids:
                gflat = g.rearrange("a c -> (a c)")
                starts = list(range(0, whole - step + 1, step))
                if starts[-1] + step < whole:
                    starts.append(whole - step)  # overlapped tail, re-zeroed
                for s in starts:
                    nc.sync.dma_start(
                        gflat[s:s + step].rearrange("(p m) -> p m", p=P),
                        ztile[:])

        # ---- pixel-constant tiles (shared by all cameras) ----
        wp_cm = tc.tile_pool(name="work", bufs=6)
        wp = wp_cm.__enter__()
        idl = pp.tile([P, FCAM], i32, tag="idl")
        nc.gpsimd.iota(idl[:], pattern=[[1, FCAM]], base=0, channel_multiplier=FCAM)
        idf = pp.tile([P, FCAM], f32, tag="idf")
        nc.scalar.copy(idf[:], idl[:])
        yf = pp.tile([P, FCAM], f32, tag="yf")
        xf = pp.tile([P, FCAM], f32, tag="xf")
        tmp = wp.tile([P, FCAM], f32, tag="t0")
        nc.scalar.activation(tmp[:], idf[:], ACT.Copy, bias=0.5 / W, scale=1.0 / W)
        _floor(nc, wp, yf[:], tmp[:], FCAM)
        nc.vector.scalar_tensor_tensor(out=xf[:], in0=yf[:], scalar=-float(W),
                                       in1=idf[:], op0=AL.mult, op1=AL.add)

        # ---- per-camera geometry constants ----
        cam_consts = []
        for cam in range(NCAM):
            kc = pp.tile([3, 3], f32, tag=f"kc_{cam}")
            ec = pp.tile([4, 4], f32, tag=f"ec_{cam}")
            tmc = pp.tile([4, 4], f32, tag=f"tc_{cam}")
            nc.sync.dma_start(kc[:], kmat[cam])
            nc.sync.dma_start(ec[:], emat[cam])
            nc.sync.dma_start(tmc[:], tmat_t[cam])
            m4p = psp.tile([4, 4], f32, tag="smallp")
            nc.tensor.matmul(m4p[:], lhsT=tmc[:], rhs=ec[:],
                             start=True, stop=True)
            m4 = pp.tile([4, 4], f32, tag=f"m4_{cam}")
            nc.vector.tensor_copy(m4[:], m4p[:])
            m4tp = psp.tile([4, 4], f32, tag="smallp")
            nc.tensor.transpose(out=m4tp[:], in_=m4[:], identity=ident[:4, :4])
            m4t = pp.tile([4, 4], f32, tag=f"m4t_{cam}")
            nc.vector.tensor_copy(m4t[:], m4tp[:])
            n3p = psp.tile([3, 3], f32, tag="smallp")
            nc.tensor.matmul(n3p[:], lhsT=m4t[:3, :3], rhs=kc[:],
                             start=True, stop=True)
            n3 = pp.tile([3, 3], f32, tag=f"n3_{cam}")
            nc.vector.tensor_copy(n3[:], n3p[:])
            pk = pp.tile([1, 12], f32, tag=f"pk_{cam}")
            for i in range(3):
                nc.gpsimd.dma_start(pk[:, 3 * i:3 * i + 3], n3[i:i + 1, :])
                nc.gpsimd.dma_start(pk[:, 9 + i:10 + i], m4[i:i + 1, 3:4])
            bc = pp.tile([P, 12], f32, tag=f"bc_{cam}")
            nc.gpsimd.partition_broadcast(bc[:], pk[:])
            cam_consts.append(bc)

        # ---- load depths ----
        dtile = pp.tile([P, F], f32, tag="dtile")
        for cam in range(NCAM):
            cs = slice(cam * FCAM, (cam + 1) * FCAM)
            nc.sync.dma_start(
                dtile[:, cs],
                dep[cam * HW:(cam + 1) * HW].rearrange("(p f) -> p f", p=P))

        # ---- lift: voxel linear index per token ----
        linf = pp.tile([P, F], f32, tag="linf")
        nc.vector.memset(linf[:], float(TRASH))
        for cam in range(NCAM):
            cs = slice(cam * FCAM, (cam + 1) * FCAM)
            bc = cam_consts[cam]
            d = dtile[:, cs]
            gs = []
            for i in range(3):
                a = wp.tile([P, FCAM], f32, tag="a_i")
                nc.vector.tensor_scalar(out=a[:], in0=xf[:],
                                        scalar1=bc[:, 3 * i:3 * i + 1],
                                        scalar2=None, op0=AL.mult)
                nc.vector.scalar_tensor_tensor(out=a[:], in0=yf[:],
                                               scalar=bc[:, 3 * i + 1:3 * i + 2],
                                               in1=a[:], op0=AL.mult, op1=AL.add)
                nc.vector.tensor_scalar(out=a[:], in0=a[:],
                                        scalar1=bc[:, 3 * i + 2:3 * i + 3],
                                        scalar2=None, op0=AL.add)
                e = wp.tile([P, FCAM], f32, tag="e_i")
                nc.vector.tensor_tensor(out=e[:], in0=a[:], in1=d, op=AL.mult)
                nc.vector.tensor_scalar(out=e[:], in0=e[:],
                                        scalar1=bc[:, 9 + i:10 + i],
                                        scalar2=None, op0=AL.add)
                mid = (X / 2.0, Y / 2.0, Z / 2.0)[i]
                g = wp.tile([P, FCAM], f32, tag=f"g_{i}")
                nc.scalar.activation(g[:], e[:], ACT.Copy, bias=mid, scale=2.0)
                gs.append(g)
            gx, gy, gz = gs
            v = wp.tile([P, FCAM], f32, tag="v")
            nc.vector.tensor_scalar(out=v[:], in0=gx[:], scalar1=-1.0,
                                    scalar2=None, op0=AL.is_gt)
            nc.vector.scalar_tensor_tensor(out=v[:], in0=gx[:], scalar=float(X),
                                           in1=v[:], op0=AL.is_lt,
                                           op1=AL.logical_and)
            for gg, bound in ((gy, float(Y)), (gz, float(Z))):
                v2 = wp.tile([P, FCAM], f32, tag="v2")
                nc.vector.tensor_scalar(out=v2[:], in0=gg[:], scalar1=-1.0,
                                        scalar2=None, op0=AL.is_gt)
                nc.vector.scalar_tensor_tensor(out=v2[:], in0=gg[:], scalar=bound,
                                               in1=v2[:], op0=AL.is_lt,
                                               op1=AL.logical_and)
                nc.vector.tensor_tensor(out=v[:], in0=v[:], in1=v2[:],
                                        op=AL.logical_and)
            fx = wp.tile([P, FCAM], f32, tag="fx")
            fy = wp.tile([P, FCAM], f32, tag="fy")
            fz = wp.tile([P, FCAM], f32, tag="fz")
            _floor(nc, wp, fx[:], gx[:], FCAM)
            _floor(nc, wp, fy[:], gy[:], FCAM)
            _floor(nc, wp, fz[:], gz[:], FCAM)
            for ft in (fx, fy, fz):
                nc.vector.tensor_scalar(out=ft[:], in0=ft[:], scalar1=0.0,
                                        scalar2=255.0, op0=AL.max, op1=AL.min)
            lf = linf[:, cs]
            nc.vector.scalar_tensor_tensor(out=lf, in0=fz[:], scalar=float(X),
                                           in1=fx[:], op0=AL.mult, op1=AL.add)
            nc.vector.scalar_tensor_tensor(out=lf, in0=lf, scalar=float(Y),
                                           in1=fy[:], op0=AL.mult, op1=AL.add)
            # blend invalid -> TRASH :  lin = TRASH + v*(lin-TRASH)
            nc.vector.tensor_scalar(out=lf, in0=lf, scalar1=-float(TRASH),
                                    scalar2=None, op0=AL.add)
            nc.vector.tensor_tensor(out=lf, in0=lf, in1=v[:], op=AL.mult)
            nc.vector.tensor_scalar(out=lf, in0=lf, scalar1=float(TRASH),
                                    scalar2=None, op0=AL.add)

        # ---- stage lin columns to DRAM (chunk-major) ----
        NBLK = (F + P - 1) // P
        lin_t = pp.tile([P, NBLK, P], f32, tag="lin_t")
        for blk in range(NBLK):
            w = min(P, F - blk * P)
            ltp = psp.tile([P, P], f32, tag="tp")
            nc.tensor.transpose(out=ltp[:w, :], in_=linf[:, blk * P:blk * P + w],
                                identity=ident[:])
            nc.vector.tensor_copy(lin_t[:w, blk, :], ltp[:w, :])
            nc.sync.dma_start(lin_dram[blk * P:blk * P + w, :],
                              lin_t[:w, blk, :])

        # ---- scatter: per-chunk dedup + CCE-add over R replica chains ----
        def chunk_body(fidx, rep):
            col_f = wp.tile([P, 1], f32, tag="colf")
            nc.sync.dma_start(col_f[:], lin_dram[fidx][:, None])
            payc = wp.tile([P, CG], f32, tag="payc")
            nc.sync.dma_start(payc[:], feat[bass.ts(fidx, P), :])
            tp = psp.tile([P, P], f32, tag="tp")
            nc.tensor.transpose(out=tp[:], in_=col_f[:].to_broadcast([P, P]),
                                identity=ident[:])
            eq = wp.tile([P, P], f32, tag="eq")
            nc.vector.tensor_tensor(out=eq[:], in0=col_f[:].to_broadcast([P, P]),
                                    in1=tp[:], op=AL.is_equal)
            earlier = wp.tile([P, P], f32, tag="earlier")
            nc.vector.tensor_tensor(out=earlier[:], in0=eq[:], in1=lowmask[:],
                                    op=AL.mult)
            ecnt = wp.tile([P, 1], f32, tag="ecnt")
            nc.vector.tensor_reduce(out=ecnt[:], in_=earlier[:],
                                    axis=mybir.AxisListType.X, op=AL.add)
            totp = psp.tile([P, CG], f32, tag="totp")
            nc.tensor.matmul(totp[:], lhsT=eq[:], rhs=payc[:],
                             start=True, stop=True)
            tot = wp.tile([P, CG], GD, tag="tot")
            nc.scalar.copy(tot[:], totp[:])
            # di = first ? col : trash_col     (first = ecnt==0)
            dfa = wp.tile([P, 1], f32, tag="dfa")
            nc.vector.tensor_scalar(out=dfa[:], in0=col_f[:],
                                    scalar1=trash_col[:], scalar2=None,
                                    op0=AL.subtract)
            dfb = wp.tile([P, 1], f32, tag="dfb")
            nc.vector.scalar_tensor_tensor(out=dfb[:], in0=ecnt[:], scalar=0.0,
                                           in1=dfa[:], op0=AL.is_equal,
                                           op1=AL.mult)
            nc.vector.tensor_scalar(out=dfb[:], in0=dfb[:],
                                    scalar1=trash_col[:], scalar2=None,
                                    op0=AL.add)
            di = wp.tile([P, 1], i32, tag="di")
            nc.scalar.copy(di[:], dfb[:])
            nc.gpsimd.indirect_dma_start(
                out=grids[rep],
                out_offset=bass.IndirectOffsetOnAxis(ap=di[:], axis=0),
                in_=tot[:], in_offset=None,
                compute_op=AL.add)

        def unrollable_body(iv0, unroll):
            for i in range(unroll):
                chunk_body(iv0 + i, i % R)

        tc.For_i_unrolled_general(start=0, end=F, step=1,
                                  unrollable_body=unrollable_body,
                                  max_unroll=UNROLL)

        # ---- merge replicas + z-max + store ----
        wp_cm.__exit__(None, None, None)
        SL = X * Y
        HF = SL // P // 2  # half-slice free length (256)
        with tc.tile_pool(name="merge", bufs=2) as mp, \
             tc.tile_pool(name="mload", bufs=4) as lp:
            for h in range(2):
                acc = mp.tile([P, HF, CG], GD, tag="acc")
                for z in range(Z):
                    rows = slice(z * SL + h * (SL // 2),
                                 z * SL + (h + 1) * (SL // 2))
                    sz = mp.tile([P, HF, CG], GD, tag="sz")
                    for r in range(R):
                        ta = lp.tile([P, HF, CG], GD, tag="ta")
                        nc.sync.dma_start(
                            ta[:],
                            grids[r][rows, :].rearrange("(p f) c -> p f c", p=P))
                        if r == 0:
                            szin = ta
                        else:
                            nc.vector.tensor_tensor(out=sz[:], in0=szin[:],
                                                    in1=ta[:], op=AL.add)
                            szin = sz
                    if z == 0:
                        nc.vector.tensor_copy(acc[:], szin[:])
                    else:
                        nc.vector.tensor_tensor(out=acc[:], in0=acc[:],
                                                in1=szin[:], op=AL.max)
                accf = mp.tile([P, HF, CG], f32, tag="accf")
                nc.vector.tensor_copy(accf[:], acc[:])
                orows = slice(h * (SL // 2), (h + 1) * (SL // 2))
                nc.sync.dma_start(
                    out[orows, :].rearrange("(p f) c -> p f c", p=P), accf[:])


def kernel(feat_maps, depths, K, E, T):
    global _prog_cache
    feat_maps = np.asarray(feat_maps, np.float32)
    depths = np.asarray(depths, np.float32)
    K = np.asarray(K, np.float32)
    E = np.asarray(E, np.float32)
    T = np.asarray(T, np.float32)

    if _prog_cache is None:
        _prog_cache = _build_program()
    nc = _prog_cache

    in_maps = []
    for core in range(8):
        b, cg = core // 4, core % 4
        ch = slice(cg * CG, (cg + 1) * CG)
        frows = np.concatenate([
            np.ascontiguousarray(
                feat_maps[b * N + n, ch].transpose(1, 2, 0).reshape(HW, CG)
                .reshape(P, FCAM, CG).transpose(1, 0, 2).reshape(HW, CG))
            for n in range(N)], axis=0)
        cams = slice(b * N, (b + 1) * N)
        in_maps.append({
            "feat": frows,
            "dep": np.ascontiguousarray(depths[b].reshape(NTOK)),
            "kmat": np.ascontiguousarray(K[cams]),
            "emat": np.ascontiguousarray(E[cams]),
            "tmat_t": np.ascontiguousarray(T[cams].transpose(0, 2, 1)),
        })

    _tr = bool(os.environ.get("KBEV_TRACE"))
    res = bass_utils.run_bass_kernel_spmd(
        nc, in_maps, core_ids=list(range(8)), trace=_tr,
        trace_cores=(list(range(8)) if os.environ.get("KBEV_TRACE_ALL")
                     else [0]) if _tr else None)
    global last_result
    last_result = res
    outp = np.zeros((1, B * C, X, Y), np.float32)
    for core in range(8):
        b, cg = core // 4, core % 4
        o = res.results[core]["out"].reshape(X, Y, CG)
        outp[0, b * C + cg * CG:b * C + (cg + 1) * CG] = \
            o.transpose(2, 0, 1)[:, ::-1, ::-1]
    return outp


# revision 7
# speedup vs baseline: 2.4958x; 1.2771x over previous
"""BEVLiftNet Trainium2 kernel (V2: CCE-add scatter, replica chains).

Sharding: 8 cores = 2 batches x 4 channel-groups (16 ch each).
Each core lifts all 4 cameras of its batch (depths -> voxel ids), then
scatter-adds 16-channel feature rows into R replica DRAM grids via
indirect DMA with CCE-add (compute_op=add).  One call per 128-token
chunk; within-chunk duplicate rows are pre-combined with an eq-matrix
matmul and redirected to per-partition-unique trash rows, so every
CCE call carries unique row indices (concurrent CCE RMWs to the same
address lose updates -- measured).  Chunks round-robin over R replica
tensors so R WAW chains overlap and gpsimd issues calls back-to-back.
A final pass sums the replicas, max-reduces over Z, and stores
[X*Y, 16] f32; the host reassembles the [1, B*C, X, Y] output.
"""

import os
import sys

sys.path.insert(0, "/opt/trn_rl_repo")

import numpy as np

import concourse.bacc as bacc
import concourse.bass as bass
import concourse.mybir as mybir
import concourse.tile as tile
from concourse import bass_utils
from concourse.masks import make_identity

B, N, C, H, W = 2, 4, 64, 128, 352
X, Y, Z = 256, 256, 8
CG = 16           # channels per core
NCAM = 4          # cameras per core (one batch)
HW = H * W        # 45056 pixels per camera
NTOK = NCAM * HW  # 180224 tokens per core
P = 128
FCAM = HW // P    # 352 chunk-columns per camera
F = NCAM * FCAM   # 1408 chunk-columns total
NROWS = Z * X * Y          # 524288 voxel rows per grid
TRASH = NROWS              # row for first invalid token
GROWS = NROWS + 1 + P      # + shared trash + per-partition trash
f32 = mybir.dt.float32
bf16 = mybir.dt.bfloat16
i32 = mybir.dt.int32
AL = mybir.AluOpType
ACT = mybir.ActivationFunctionType

R = int(os.environ.get("KBEV_R", "4"))           # replica grids
GD = bf16 if os.environ.get("KBEV_GD", "bf16") == "bf16" else f32
UNROLL = int(os.environ.get("KBEV_UNROLL", "32"))

_prog_cache = None


def _build_program():
    nc = bacc.Bacc("TRN2", target_bir_lowering=False, debug=False)
    feat = nc.dram_tensor("feat", [NTOK, CG], f32, kind="ExternalInput")
    dep = nc.dram_tensor("dep", [NTOK], f32, kind="ExternalInput")
    kmat = nc.dram_tensor("kmat", [NCAM, 3, 3], f32, kind="ExternalInput")
    emat = nc.dram_tensor("emat", [NCAM, 4, 4], f32, kind="ExternalInput")
    tmat_t = nc.dram_tensor("tmat_t", [NCAM, 4, 4], f32, kind="ExternalInput")
    out = nc.dram_tensor("out", [X * Y, CG], f32, kind="ExternalOutput")
    grids = [nc.dram_tensor(f"grid{r}", [GROWS, CG], GD, kind="Internal")
             for r in range(R)]
    lin_dram = nc.dram_tensor("lin_dram", [F, P], f32, kind="Internal")
    with tile.TileContext(nc) as tc:
        _emit(tc, feat.ap(), dep.ap(), kmat.ap(), emat.ap(), tmat_t.ap(),
              out.ap(), [g.ap() for g in grids], lin_dram.ap())
    nc.compile()
    return nc


def _floor(nc, wp, out_t, in_t, n):
    """out = floor(in), robust to any f32->i32 rounding mode."""
    ii = wp.tile([P, n], i32, tag="fl_i")
    nc.vector.tensor_copy(ii[:], in_t)
    ff = wp.tile([P, n], f32, tag="fl_f")
    nc.vector.tensor_copy(ff[:], ii[:])
    gt = wp.tile([P, n], f32, tag="fl_g")
    nc.vector.tensor_tensor(out=gt[:], in0=ff[:], in1=in_t, op=AL.is_gt)
    nc.vector.tensor_tensor(out=out_t, in0=ff[:], in1=gt[:], op=AL.subtract)


def _emit(tc, feat, dep, kmat, emat, tmat_t, out, grids, lin_dram):
    nc = tc.nc

    with tc.tile_pool(name="persist", bufs=1) as pp, \
         tc.tile_pool(name="psum", bufs=2, space="PSUM") as psp:

        ident = pp.tile([P, P], f32, tag="ident")
        make_identity(nc, ident[:])
        # lowmask[p, k] = 1.0 where k < p   (strictly lower along free dim)
        iot = pp.tile([P, P], i32, tag="iotpk")
        nc.gpsimd.iota(iot[:], pattern=[[1, P]], base=0, channel_multiplier=-1)
        lowmask = pp.tile([P, P], f32, tag="lowmask")
        nc.vector.tensor_scalar(out=lowmask[:], in0=iot[:], scalar1=0,
                                scalar2=None, op0=AL.is_lt)
        # trash_col[p] = TRASH + 1 + p  (per-partition-unique trash rows)
        piot = pp.tile([P, 1], i32, tag="piot")
        nc.gpsimd.iota(piot[:], pattern=[[0, 1]], base=TRASH + 1,
                       channel_multiplier=1)
        trash_col = pp.tile([P, 1], f32, tag="trashc")
        nc.scalar.copy(trash_col[:], piot[:])

        # ---- zero the replica grids (HWDGE, overlaps with lift) ----
        with tc.tile_pool(name="zpool", bufs=1) as zp:
            ztile = zp.tile([P, 4096], GD, tag="zz")
            nc.vector.memset(ztile[:], 0.0)
            whole = GROWS * CG
            step = P * 4096
            for g in grids:
                gflat = g.rearrange("a c -> (a c)")
                starts = list(range(0, whole - step + 1, step))
                if starts[-1] + step < whole:
                    starts.append(whole - step)  # overlapped tail, re-zeroed
                for s in starts:
                    nc.sync.dma_start(
                        gflat[s:s + step].rearrange("(p m) -> p m", p=P),
                        ztile[:])

        # ---- pixel-constant tiles (shared by all cameras) ----
        wp_cm = tc.tile_pool(name="work", bufs=6)
        wp = wp_cm.__enter__()
        idl = pp.tile([P, FCAM], i32, tag="idl")
        nc.gpsimd.iota(idl[:], pattern=[[1, FCAM]], base=0, channel_multiplier=FCAM)
        idf = pp.tile([P, FCAM], f32, tag="idf")
        nc.scalar.copy(idf[:], idl[:])
        yf = pp.tile([P, FCAM], f32, tag="yf")
        xf = pp.tile([P, FCAM], f32, tag="xf")
        tmp = wp.tile([P, FCAM], f32, tag="t0")
        nc.scalar.activation(tmp[:], idf[:], ACT.Copy, bias=0.5 / W, scale=1.0 / W)
        _floor(nc, wp, yf[:], tmp[:], FCAM)
        nc.vector.scalar_tensor_tensor(out=xf[:], in0=yf[:], scalar=-float(W),
                                       in1=idf[:], op0=AL.mult, op1=AL.add)

        # ---- per-camera geometry constants ----
        cam_consts = []
        for cam in range(NCAM):
            kc = pp.tile([3, 3], f32, tag=f"kc_{cam}")
            ec = pp.tile([4, 4], f32, tag=f"ec_{cam}")
            tmc = pp.tile([4, 4], f32, tag=f"tc_{cam}")
            nc.sync.dma_start(kc[:], kmat[cam])
            nc.sync.dma_start(ec[:], emat[cam])
            nc.sync.dma_start(tmc[:], tmat_t[cam])
            m4p = psp.tile([4, 4], f32, tag="smallp")
            nc.tensor.matmul(m4p[:], lhsT=tmc[:], rhs=ec[:],
                             start=True, stop=True)
            m4 = pp.tile([4, 4], f32, tag=f"m4_{cam}")
            nc.vector.tensor_copy(m4[:], m4p[:])
            m4tp = psp.tile([4, 4], f32, tag="smallp")
            nc.tensor.transpose(out=m4tp[:], in_=m4[:], identity=ident[:4, :4])
            m4t = pp.tile([4, 4], f32, tag=f"m4t_{cam}")
            nc.vector.tensor_copy(m4t[:], m4tp[:])
            n3p = psp.tile([3, 3], f32, tag="smallp")
            nc.tensor.matmul(n3p[:], lhsT=m4t[:3, :3], rhs=kc[:],
                             start=True, stop=True)
            n3 = pp.tile([3, 3], f32, tag=f"n3_{cam}")
            nc.vector.tensor_copy(n3[:], n3p[:])
            pk = pp.tile([1, 12], f32, tag=f"pk_{cam}")
            for i in range(3):
                nc.gpsimd.dma_start(pk[:, 3 * i:3 * i + 3], n3[i:i + 1, :])
                nc.gpsimd.dma_start(pk[:, 9 + i:10 + i], m4[i:i + 1, 3:4])
            bc = pp.tile([P, 12], f32, tag=f"bc_{cam}")
            nc.gpsimd.partition_broadcast(bc[:], pk[:])
            cam_consts.append(bc)

        # ---- load depths ----
        dtile = pp.tile([P, F], f32, tag="dtile")
        for cam in range(NCAM):
            cs = slice(cam * FCAM, (cam + 1) * FCAM)
            nc.sync.dma_start(
                dtile[:, cs],
                dep[cam * HW:(cam + 1) * HW].rearrange("(p f) -> p f", p=P))

        # ---- lift: voxel linear index per token ----
        linf = pp.tile([P, F], f32, tag="linf")
        nc.vector.memset(linf[:], float(TRASH))
        for cam in range(NCAM):
            cs = slice(cam * FCAM, (cam + 1) * FCAM)
            bc = cam_consts[cam]
            d = dtile[:, cs]
            gs = []
            for i in range(3):
                a = wp.tile([P, FCAM], f32, tag="a_i")
                nc.vector.tensor_scalar(out=a[:], in0=xf[:],
                                        scalar1=bc[:, 3 * i:3 * i + 1],
                                        scalar2=None, op0=AL.mult)
                nc.vector.scalar_tensor_tensor(out=a[:], in0=yf[:],
                                               scalar=bc[:, 3 * i + 1:3 * i + 2],
                                               in1=a[:], op0=AL.mult, op1=AL.add)
                nc.vector.tensor_scalar(out=a[:], in0=a[:],
                                        scalar1=bc[:, 3 * i + 2:3 * i + 3],
                                        scalar2=None, op0=AL.add)
                e = wp.tile([P, FCAM], f32, tag="e_i")
                nc.vector.tensor_tensor(out=e[:], in0=a[:], in1=d, op=AL.mult)
                nc.vector.tensor_scalar(out=e[:], in0=e[:],
                                        scalar1=bc[:, 9 + i:10 + i],
                                        scalar2=None, op0=AL.add)
                mid = (X / 2.0, Y / 2.0, Z / 2.0)[i]
                g = wp.tile([P, FCAM], f32, tag=f"g_{i}")
                nc.scalar.activation(g[:], e[:], ACT.Copy, bias=mid, scale=2.0)
                gs.append(g)
            gx, gy, gz = gs
            v = wp.tile([P, FCAM], f32, tag="v")
            nc.vector.tensor_scalar(out=v[:], in0=gx[:], scalar1=-1.0,
                                    scalar2=None, op0=AL.is_gt)
            nc.vector.scalar_tensor_tensor(out=v[:], in0=gx[:], scalar=float(X),
                                           in1=v[:], op0=AL.is_lt,
                                           op1=AL.logical_and)
            for gg, bound in ((gy, float(Y)), (gz, float(Z))):
                v2 = wp.tile([P, FCAM], f32, tag="v2")
                nc.vector.tensor_scalar(out=v2[:], in0=gg[:], scalar1=-1.0,
                                        scalar2=None, op0=AL.is_gt)
                nc.vector.scalar_tensor_tensor(out=v2[:], in0=gg[:], scalar=bound,
                                               in1=v2[:], op0=AL.is_lt,
                                               op1=AL.logical_and)
                nc.vector.tensor_tensor(out=v[:], in0=v[:], in1=v2[:],
                                        op=AL.logical_and)
            fx = wp.tile([P, FCAM], f32, tag="fx")
            fy = wp.tile([P, FCAM], f32, tag="fy")
            fz = wp.tile([P, FCAM], f32, tag="fz")
            _floor(nc, wp, fx[:], gx[:], FCAM)
            _floor(nc, wp, fy[:], gy[:], FCAM)
            _floor(nc, wp, fz[:], gz[:], FCAM)
            for ft in (fx, fy, fz):
                nc.vector.tensor_scalar(out=ft[:], in0=ft[:], scalar1=0.0,
                                        scalar2=255.0, op0=AL.max, op1=AL.min)
            lf = linf[:, cs]
            nc.vector.scalar_tensor_tensor(out=lf, in0=fz[:], scalar=float(X),
                                           in1=fx[:], op0=AL.mult, op1=AL.add)
            nc.vector.scalar_tensor_tensor(out=lf, in0=lf, scalar=float(Y),
                                           in1=fy[:], op0=AL.mult, op1=AL.add)
            # blend invalid -> TRASH :  lin = TRASH + v*(lin-TRASH)
            nc.vector.tensor_scalar(out=lf, in0=lf, scalar1=-float(TRASH),
                                    scalar2=None, op0=AL.add)
            nc.vector.tensor_tensor(out=lf, in0=lf, in1=v[:], op=AL.mult)
            nc.vector.tensor_scalar(out=lf, in0=lf, scalar1=float(TRASH),
                                    scalar2=None, op0=AL.add)

        # ---- stage lin columns to DRAM (chunk-major) ----
        NBLK = (F + P - 1) // P
        lin_t = pp.tile([P, NBLK, P], f32, tag="lin_t")
        for blk in range(NBLK):
            w = min(P, F - blk * P)
            ltp = psp.tile([P, P], f32, tag="tp")
            nc.tensor.transpose(out=ltp[:w, :], in_=linf[:, blk * P:blk * P + w],
                                identity=ident[:])
            nc.vector.tensor_copy(lin_t[:w, blk, :], ltp[:w, :])
            nc.sync.dma_start(lin_dram[blk * P:blk * P + w, :],
                              lin_t[:w, blk, :])

        # ---- scatter: per-chunk dedup + CCE-add over R replica chains ----
        def chunk_body(fidx, rep):
            col_f = wp.tile([P, 1], f32, tag="colf")
            nc.sync.dma_start(col_f[:], lin_dram[fidx][:, None])
            payc = wp.tile([P, CG], f32, tag="payc")
            nc.scalar.dma_start(payc[:], feat[bass.ts(fidx, P), :])
            tp = psp.tile([P, P], f32, tag="tp")
            nc.tensor.transpose(out=tp[:], in_=col_f[:].to_broadcast([P, P]),
                                identity=ident[:])
            eq = wp.tile([P, P], f32, tag="eq")
            nc.vector.tensor_tensor(out=eq[:], in0=col_f[:].to_broadcast([P, P]),
                                    in1=tp[:], op=AL.is_equal)
            earlier = wp.tile([P, P], f32, tag="earlier")
            nc.vector.tensor_tensor(out=earlier[:], in0=eq[:], in1=lowmask[:],
                                    op=AL.mult)
            ecnt = wp.tile([P, 1], f32, tag="ecnt")
            nc.vector.tensor_reduce(out=ecnt[:], in_=earlier[:],
                                    axis=mybir.AxisListType.X, op=AL.add)
            totp = psp.tile([P, CG], f32, tag="totp")
            nc.tensor.matmul(totp[:], lhsT=eq[:], rhs=payc[:],
                             start=True, stop=True)
            tot = wp.tile([P, CG], GD, tag="tot")
            nc.scalar.copy(tot[:], totp[:])
            # di = first ? col : trash_col     (first = ecnt==0)
            dfa = wp.tile([P, 1], f32, tag="dfa")
            nc.vector.tensor_scalar(out=dfa[:], in0=col_f[:],
                                    scalar1=trash_col[:], scalar2=None,
                                    op0=AL.subtract)
            dfb = wp.tile([P, 1], f32, tag="dfb")
            nc.vector.scalar_tensor_tensor(out=dfb[:], in0=ecnt[:], scalar=0.0,
                                           in1=dfa[:], op0=AL.is_equal,
                                           op1=AL.mult)
            nc.vector.tensor_scalar(out=dfb[:], in0=dfb[:],
                                    scalar1=trash_col[:], scalar2=None,
                                    op0=AL.add)
            di = wp.tile([P, 1], i32, tag="di")
            nc.scalar.copy(di[:], dfb[:])
            nc.gpsimd.indirect_dma_start(
                out=grids[rep],
                out_offset=bass.IndirectOffsetOnAxis(ap=di[:], axis=0),
                in_=tot[:], in_offset=None,
                compute_op=AL.add)

        def unrollable_body(iv0, unroll):
            for i in range(unroll):
                chunk_body(iv0 + i, i % R)

        tc.For_i_unrolled_general(start=0, end=F, step=1,
                                  unrollable_body=unrollable_body,
                                  max_unroll=UNROLL)

        # ---- merge replicas + z-max + store ----
        wp_cm.__exit__(None, None, None)
        SL = X * Y
        HF = SL // P // 2  # half-slice free length (256)
        with tc.tile_pool(name="merge", bufs=2) as mp, \
             tc.tile_pool(name="mload", bufs=4) as lp:
            for h in range(2):
                acc = mp.tile([P, HF, CG], GD, tag="acc")
                for z in range(Z):
                    rows = slice(z * SL + h * (SL // 2),
                                 z * SL + (h + 1) * (SL // 2))
                    sz = mp.tile([P, HF, CG], GD, tag="sz")
                    for r in range(R):
                        ta = lp.tile([P, HF, CG], GD, tag="ta")
                        nc.sync.dma_start(
                            ta[:],
                            grids[r][rows, :].rearrange("(p f) c -> p f c", p=P))
                        if r == 0:
                            szin = ta
                        else:
                            nc.vector.tensor_tensor(out=sz[:], in0=szin[:],
                                                    in1=ta[:], op=AL.add)
                            szin = sz
                    if z == 0:
                        nc.vector.tensor_copy(acc[:], szin[:])
                    else:
                        nc.vector.tensor_tensor(out=acc[:], in0=acc[:],
                                                in1=szin[:], op=AL.max)
                accf = mp.tile([P, HF, CG], f32, tag="accf")
                nc.vector.tensor_copy(accf[:], acc[:])
                orows = slice(h * (SL // 2), (h + 1) * (SL // 2))
                nc.sync.dma_start(
                    out[orows, :].rearrange("(p f) c -> p f c", p=P), accf[:])


def kernel(feat_maps, depths, K, E, T):
    global _prog_cache
    feat_maps = np.asarray(feat_maps, np.float32)
    depths = np.asarray(depths, np.float32)
    K = np.asarray(K, np.float32)
    E = np.asarray(E, np.float32)
    T = np.asarray(T, np.float32)

    if _prog_cache is None:
        _prog_cache = _build_program()
    nc = _prog_cache

    in_maps = []
    for core in range(8):
        b, cg = core // 4, core % 4
        ch = slice(cg * CG, (cg + 1) * CG)
        frows = np.concatenate([
            np.ascontiguousarray(
                feat_maps[b * N + n, ch].transpose(1, 2, 0).reshape(HW, CG)
                .reshape(P, FCAM, CG).transpose(1, 0, 2).reshape(HW, CG))
            for n in range(N)], axis=0)
        cams = slice(b * N, (b + 1) * N)
        in_maps.append({
            "feat": frows,
            "dep": np.ascontiguousarray(depths[b].reshape(NTOK)),
            "kmat": np.ascontiguousarray(K[cams]),
            "emat": np.ascontiguousarray(E[cams]),
            "tmat_t": np.ascontiguousarray(T[cams].transpose(0, 2, 1)),
        })

    _tr = bool(os.environ.get("KBEV_TRACE"))
    res = bass_utils.run_bass_kernel_spmd(
        nc, in_maps, core_ids=list(range(8)), trace=_tr,
        trace_cores=(list(range(8)) if os.environ.get("KBEV_TRACE_ALL")
                     else [0]) if _tr else None)
    global last_result
    last_result = res
    outp = np.zeros((1, B * C, X, Y), np.float32)
    for core in range(8):
        b, cg = core // 4, core % 4
        o = res.results[core]["out"].reshape(X, Y, CG)
        outp[0, b * C + cg * CG:b * C + (cg + 1) * CG] = \
            o.transpose(2, 0, 1)[:, ::-1, ::-1]
    return outp
